# revision 3
# baseline (speedup 1.0000x reference)
"""Trainium2 Bass kernel for nn_DiagonalSelectiveSSM.

Math (reference):
    a = tanh(a_logit); a_safe = sign-clamped to |a|>=1e-4
    g = sigmoid(x @ W^T + gate_b)
    u = b * g * x
    pows[t] = cumprod(a_safe) (fp32, underflows to exact 0 under FTZ)
    v = u / (pows + 1e-12); s = cumsum(v) * pows; h = c*s + d*x

Identity: s_t = a_safe * s_{t-1} + w_t  with  w_t = u_t * F_t and
F_t = pows_t / (pows_t + 1e-12). For a<0 channels F has catastrophic-
cancellation *spikes* (|F| up to ~1e3-1e4 where pows ~ -1e-12) which
dominate the output norm, so F must be computed from the exact fp32
cumprod bit pattern: the device rebuilds the pows tile as
    p = pstart[g,j] (exact fp32 bits at each block start) * apow[tau]
and F = p*(p+K) / ((p+K)^2 + tau^2)   (zero-safe soft reciprocal,
tau=1e-17 caps |F| at ~2.5e4; exact to ~1e-6 for |p+K| >> tau).

Once pows underflows the reference output is exactly 0 -> dead tiles are
skipped (runtime pre-zeros outputs). b/c/d generality is a host epilogue
(s is linear in u: h = (c*b)*s_unit + d*x).

Sharding: 8 cores = 4 sequences x 2 channel-halves. Channels within a
half are sorted by |a_safe| so liveness is uniform per 128-channel
group. x is shipped once per core as fp16, host-packed to [P, j, KC*TB]
so each block load is 128 partitions x 8KB contiguous, and chunk g of
the contraction holds exactly the core's group-g output channels (the
elementwise x is reused from the matmul tile - no separate x stream).
"""

import os

import numpy as np

B, T, D = 4, 8192, 1024
E = D // 2          # channels per core
P = 128             # partitions
NG = E // P         # channel groups per core
TB = 512            # time-block (one PSUM bank of fp32)
NT = T // TB
KC = D // P         # contraction chunks
N_CORES = 8
FP32_MIN_NORMAL = np.float32(1.1754944e-38)
KREG = 1e-12        # the reference's regularizer
TAU2 = float(np.float32(1e-17) ** 2)
MM_DTYPE = os.environ.get("KERNEL_MM_DTYPE", "fp16")
OUT_DTYPE = os.environ.get("KERNEL_OUT_DTYPE", "f32")

_prog_cache = {}


def _mm_cast(a):
    if MM_DTYPE == "f32r":
        b = np.ascontiguousarray(a, np.float32).view(np.uint32)
        lsb = (b >> 12) & 1
        return ((b + 0x7FF + lsb) & 0xFFFFF000).view(np.float32)
    if MM_DTYPE == "fp16":
        return np.ascontiguousarray(a).astype(np.float16)
    import ml_dtypes

    return np.ascontiguousarray(a).astype(ml_dtypes.bfloat16)


# ---------------------------------------------------------------- program
def _build_program(live, repeat=1, mode="full"):
    """live: tuple of NG ints - per sorted-channel-group live t-block count
    (identical across cores: union). Returns compiled Bacc program.
    repeat>1 wraps the whole body in a hardware loop (benchmarking only).
    mode: "full" | "dma" (loads/stores only) | "compute" (no bulk DMA)."""
    import concourse.tile as tile
    from concourse import bacc, mybir

    f32 = mybir.dt.float32
    mmdt = {
        "f32r": mybir.dt.float32r,
        "fp16": mybir.dt.float16,
        "bf16": mybir.dt.bfloat16,
    }[MM_DTYPE]
    outdt = f32 if OUT_DTYPE == "f32" else mybir.dt.float16
    Alu = mybir.AluOpType
    Act = mybir.ActivationFunctionType

    L0 = max(live)

    nc = bacc.Bacc(
        "TRN2",
        target_bir_lowering=False,
        debug=False,
        enable_asserts=False,
        num_devices=N_CORES,
    )

    xp_d = nc.dram_tensor("xp", [P, L0, KC, TB], mmdt, kind="ExternalInput").ap()
    wT_d = nc.dram_tensor("wT", [D, E], mmdt, kind="ExternalInput").ap()
    av_d = nc.dram_tensor("av", [P, NG], f32, kind="ExternalInput").ap()
    gb_d = nc.dram_tensor("gbv", [P, NG], f32, kind="ExternalInput").ap()
    ap_d = nc.dram_tensor("apw", [P, NG * TB], f32, kind="ExternalInput").ap()
    ps_d = nc.dram_tensor("pst", [P, NG * NT], f32, kind="ExternalInput").ap()
    h_d = nc.dram_tensor("h", [E, T], outdt, kind="ExternalOutput").ap()

    with tile.TileContext(nc) as tc:
        with (
            tc.tile_pool(name="const", bufs=1) as const,
            tc.tile_pool(name="wpool", bufs=1) as wpool,
            tc.tile_pool(name="xk", bufs=4) as xkpool,
            tc.tile_pool(name="elw", bufs=4) as elw,
            tc.tile_pool(name="spool", bufs=3) as spool,
            tc.tile_pool(name="psum", bufs=4, space="PSUM") as pspool,
        ):
            wk = []
            for k in range(KC):
                t = wpool.tile([P, E], mmdt, tag=f"w{k}")
                nc.sync.dma_start(t[:], wT_d[k * P : (k + 1) * P, :])
                wk.append(t)
            av = const.tile([P, NG], f32)
            nc.sync.dma_start(av[:], av_d[:])
            gb = const.tile([P, NG], f32)
            nc.sync.dma_start(gb[:], gb_d[:])
            apw = const.tile([P, NG * TB], f32)
            nc.sync.dma_start(apw[:], ap_d[:])
            pst = const.tile([P, NG * NT], f32)
            nc.sync.dma_start(pst[:], ps_d[:])
            ones = const.tile([P, TB], f32)
            nc.vector.memset(ones[:], 1.0)
            abc = []
            for g in range(NG):
                t = const.tile([P, TB], f32, tag=f"abc{g}")
                nc.vector.tensor_scalar_mul(t[:], ones[:], av[:, g : g + 1])
                abc.append(t)

            def body():
                prev_s = [None] * NG
                for j in range(L0):
                    ts = slice(j * TB, (j + 1) * TB)
                    xkb = xkpool.tile([P, KC, TB], mmdt, tag="xkb")
                    if mode != "compute" or j == 0:
                        nc.sync.dma_start(xkb[:], xp_d[:, j])
                    for g in range(NG):
                        es = slice(g * P, (g + 1) * P)
                        gs = slice(g * TB, (g + 1) * TB)
                        if j >= live[g]:
                            continue
                        if mode == "dma":
                            nc.scalar.dma_start(h_d[es, ts], apw[:, gs])
                            continue
                        ps = pspool.tile([P, TB], f32)
                        for k in range(KC):
                            nc.tensor.matmul(
                                ps[:],
                                wk[k][:, es],
                                xkb[:, k, :],
                                start=(k == 0),
                                stop=(k == KC - 1),
                            )
                        gt = elw.tile([P, TB], f32, tag="g")
                        nc.scalar.activation(
                            gt[:], ps[:], Act.Sigmoid,
                            bias=gb[:, g : g + 1], scale=1.0,
                        )
                        # F = p*dlt/(dlt^2 + tau^2), p = pstart*apow, dlt = p+K
                        pt = elw.tile([P, TB], f32, tag="p")
                        nc.vector.tensor_scalar_mul(
                            pt[:], apw[:, gs], pst[:, g * NT + j : g * NT + j + 1]
                        )
                        dl = elw.tile([P, TB], f32, tag="dl")
                        nc.gpsimd.tensor_scalar_add(dl[:], pt[:], KREG)
                        q = elw.tile([P, TB], f32, tag="q")
                        nc.vector.tensor_tensor(q[:], dl[:], dl[:], Alu.mult)
                        q2 = elw.tile([P, TB], f32, tag="q2")
                        nc.vector.tensor_scalar_add(q2[:], q[:], TAU2)
                        rc = elw.tile([P, TB], f32, tag="rc")
                        nc.vector.reciprocal_approx_fast(rc[:], q2[:])
                        pd = elw.tile([P, TB], f32, tag="pd")
                        nc.gpsimd.tensor_tensor(pd[:], pt[:], dl[:], Alu.mult)
                        ft = elw.tile([P, TB], f32, tag="f")
                        nc.vector.tensor_tensor(ft[:], pd[:], rc[:], Alu.mult)
                        xg = elw.tile([P, TB], f32, tag="xg")
                        nc.gpsimd.tensor_tensor(xg[:], gt[:], xkb[:, g, :], Alu.mult)
                        wt = elw.tile([P, TB], f32, tag="w")
                        nc.vector.tensor_tensor(wt[:], xg[:], ft[:], Alu.mult)
                        st = spool.tile([P, TB], outdt, tag=f"s{g}")
                        init = 0.0 if j == 0 else prev_s[g][:, TB - 1 : TB]
                        nc.vector.tensor_tensor_scan(
                            st[:], abc[g][:], wt[:], init, Alu.mult, Alu.add
                        )
                        prev_s[g] = st
                        if mode == "full":
                            nc.scalar.dma_start(h_d[es, ts], st[:])
                        # dead tiles: reference output is exactly 0 there and
                        # the runtime pre-zeros ExternalOutput buffers, so no
                        # store.

            if repeat == 1:
                body()
            else:
                with tc.For_i(0, repeat, 1):
                    body()
    nc.compile()
    return nc


# ---------------------------------------------------------------- host math
def _pows_table(a_safe):
    """fp32 sequential cumprod with FTZ, matching XLA CPU bits."""
    a_rep = np.broadcast_to(a_safe, (T, D)).astype(np.float32)
    cp = np.cumprod(a_rep[1:], axis=0, dtype=np.float32)
    pows = np.concatenate([np.ones((1, D), np.float32), cp], axis=0)
    pows[np.abs(pows) < FP32_MIN_NORMAL] = 0.0
    return pows


# ---------------------------------------------------------------- kernel
def kernel(x, a_logit, b, c, d, gate_W, gate_b):
    from concourse.bass_utils import run_bass_kernel_spmd

    x = np.ascontiguousarray(np.asarray(x, np.float32))
    a_logit = np.asarray(a_logit, np.float32)
    b = np.asarray(b, np.float32)
    c = np.asarray(c, np.float32)
    d = np.asarray(d, np.float32)
    gate_W = np.ascontiguousarray(np.asarray(gate_W, np.float32))
    gate_b = np.asarray(gate_b, np.float32)

    a = np.tanh(a_logit)
    eps = np.float32(1e-4)
    a_safe = np.where(np.abs(a) < eps, np.where(a < 0, -eps, eps), a).astype(
        np.float32
    )
    pows = _pows_table(a_safe)

    # per-half sorted channel permutations + union liveness
    perms = []
    live_by_half = []
    for half in range(2):
        idx = np.arange(half * E, (half + 1) * E)
        perm = idx[np.argsort(-np.abs(a_safe[idx]), kind="stable")]
        perms.append(perm)
        lv = []
        for g in range(NG):
            ch = perm[g * P : (g + 1) * P]
            alive = (pows[:, ch] != 0).any(axis=1).reshape(NT, TB).any(axis=1)
            nz = np.nonzero(alive)[0]
            lv.append(int(nz.max()) + 1 if nz.size else 1)
        live_by_half.append(lv)
    live = tuple(max(live_by_half[0][g], live_by_half[1][g]) for g in range(NG))
    L0 = max(live)

    key = (live, MM_DTYPE, OUT_DTYPE)
    if key not in _prog_cache:
        _prog_cache[key] = _build_program(live)
    nc = _prog_cache[key]

    in_maps = []
    for core in range(N_CORES):
        bb, half = divmod(core, 2)
        perm = perms[half]
        other = perms[1 - half]
        perm_full = np.concatenate([perm, other])              # (D,)
        xTb = x[bb].T                                          # [D, T]
        xpr = _mm_cast(xTb[perm_full, : L0 * TB])              # [D, L0*TB]
        xp = np.ascontiguousarray(
            xpr.reshape(KC, P, L0, TB).transpose(1, 2, 0, 3)
        )                                                      # [P, L0, KC, TB]
        wT = _mm_cast(np.ascontiguousarray(gate_W[np.ix_(perm, perm_full)].T))
        # apow[p, g*TB+tau] = pows[tau, perm[g*P+p]]
        apw = np.ascontiguousarray(
            pows[:TB, perm].T.reshape(NG, P, TB).transpose(1, 0, 2).reshape(
                P, NG * TB
            )
        )
        # pstart[p, g*NT+j] = pows[j*TB, perm[g*P+p]]
        pst = np.ascontiguousarray(
            pows[:: TB, perm].T.reshape(NG, P, NT).transpose(1, 0, 2).reshape(
                P, NG * NT
            )
        )
        in_maps.append(
            {
                "xp": xp,
                "wT": wT,
                "av": np.ascontiguousarray(a_safe[perm].reshape(NG, P).T),
                "gbv": np.ascontiguousarray(gate_b[perm].reshape(NG, P).T),
                "apw": apw,
                "pst": pst,
            }
        )

    global last_results, last_live, last_in_maps
    last_live = live
    last_in_maps = in_maps
    res = run_bass_kernel_spmd(nc, in_maps, core_ids=list(range(N_CORES)))
    last_results = res

    h = np.empty((B, T, D), np.float32)
    for core in range(N_CORES):
        bb, half = divmod(core, 2)
        h[bb][:, perms[half]] = res.results[core]["h"].T.astype(np.float32)

    cb = (c * b).astype(np.float32)
    if np.any(cb != 1):
        h *= cb[None, None, :]
    if np.any(d != 0):
        h += d[None, None, :] * x
    return h


last_results = None


# revision 6
# speedup vs baseline: 1.1529x; 1.1529x over previous
"""Trainium2 Bass kernel for nn_DiagonalSelectiveSSM.

Math (reference):
    a = tanh(a_logit); a_safe = sign-clamped to |a|>=1e-4
    g = sigmoid(x @ W^T + gate_b)
    u = b * g * x
    pows[t] = cumprod(a_safe) (fp32, underflows to exact 0 under FTZ)
    v = u / (pows + 1e-12); s = cumsum(v) * pows; h = c*s + d*x

Identity: s_t = a_safe * s_{t-1} + w_t  with  w_t = u_t * F_t and
F_t = pows_t / (pows_t + 1e-12). For a<0 channels F has catastrophic-
cancellation *spikes* (|F| up to ~1e4 where pows ~ -1e-12) which dominate
the output norm, so F is rebuilt on device from the exact fp32 cumprod
bits:  p = pstart[slot,j] * apow[tau],  F = p*(p+K)/((p+K)^2 + tau^2)
(zero-safe soft reciprocal; exact to ~1e-6 away from the poles).

Sharding (8 cores = 4 sequences x 2 T-halves):
  The output norm is dominated by the ~128 longest-lived channels (their
  variance is amplified by 1/(1-a^2)), and they force every t-block of x
  to be read for their gates. So per sequence the pair of cores splits T:
    role 0: long group x global blocks 0..7,  role 1: blocks 8..15.
  Both scan from state 0; the reference state at the half boundary is
  role 0's last stored column, and the host adds the exact rank-1 decay
  correction  s_true = s~ + a^(t-4095) * s(4095)  (linear recurrence)
  to role 1's long tiles during the gather pass - the chunked state
  handoff of T-sharding, applied as a host epilogue.
  The remaining 896 channels (short-lived: dead after <=2 blocks) are
  dealt between the pair as 4 slot-groups each (one is a balance dummy),
  reading the same early global blocks from dedicated xpack slots.
  Dead tiles are skipped entirely (runtime pre-zeros outputs).
b/c/d generality is a host epilogue (s is linear in u).

x ships once per core as fp16, host-packed to [P, slot, KC*TB] so each
block load is 128 partitions x 8KB contiguous; contraction chunk order
puts each slot's output channels in one chunk so the elementwise x is
reused from the matmul tile (no separate x stream).
"""

import os

import numpy as np

B, T, D = 4, 8192, 1024
P = 128             # partitions
NLB = 8             # long-group blocks per core (T/2 / TB)
NSS = 4             # short slots per core
TB = 512            # time-block (one PSUM bank of fp32)
NT = T // TB
KC = D // P         # contraction chunks
EW = P + NSS * P    # gate output channels per core (long + 4 slots) = 640
N_CORES = 8
FP32_MIN_NORMAL = np.float32(1.1754944e-38)
KREG = 1e-12
TAU2 = float(np.float32(1e-17) ** 2)
MM_DTYPE = os.environ.get("KERNEL_MM_DTYPE", "fp16")
OUT_DTYPE = os.environ.get("KERNEL_OUT_DTYPE", "f32")
LOAD_SPLIT = int(os.environ.get("KERNEL_LOAD_SPLIT", "1"))
STORE_ENG = os.environ.get("KERNEL_STORE_ENG", "scalar")

_prog_cache = {}


def _mm_cast(a):
    if MM_DTYPE == "f32r":
        b = np.ascontiguousarray(a, np.float32).view(np.uint32)
        lsb = (b >> 12) & 1
        return ((b + 0x7FF + lsb) & 0xFFFFF000).view(np.float32)
    if MM_DTYPE == "fp16":
        return np.ascontiguousarray(a).astype(np.float16)
    import ml_dtypes

    return np.ascontiguousarray(a).astype(ml_dtypes.bfloat16)


# ---------------------------------------------------------------- program
def _build_program(live, repeat=1, mode="full"):
    """live: (v3, slot_lives tuple(NSS), nsb). One program for all 8 cores;
    role differences are pure input data. repeat>1 wraps the body in a
    hardware loop (benchmarking). mode: full | dma | compute."""
    import concourse.tile as tile
    from concourse import bacc, mybir

    _tag, slot_lives, nsb = live
    f32 = mybir.dt.float32
    mmdt = {
        "f32r": mybir.dt.float32r,
        "fp16": mybir.dt.float16,
        "bf16": mybir.dt.bfloat16,
    }[MM_DTYPE]
    outdt = f32 if OUT_DTYPE == "f32" else mybir.dt.float16
    Alu = mybir.AluOpType
    Act = mybir.ActivationFunctionType

    NSLOT = NLB + nsb               # xpack slots
    NPST = NLB + NSS * nsb          # pstart columns
    HCOLS = (NLB + sum(slot_lives)) * TB

    nc = bacc.Bacc(
        "TRN2",
        target_bir_lowering=False,
        debug=False,
        enable_asserts=False,
        num_devices=N_CORES,
    )

    xp_d = nc.dram_tensor("xp", [P, NSLOT, KC, TB], mmdt, kind="ExternalInput").ap()
    wT_d = nc.dram_tensor("wT", [D, EW], mmdt, kind="ExternalInput").ap()
    av_d = nc.dram_tensor("av", [P, 1 + NSS], f32, kind="ExternalInput").ap()
    gb_d = nc.dram_tensor("gbv", [P, 1 + NSS], f32, kind="ExternalInput").ap()
    ap_d = nc.dram_tensor("apw", [P, (1 + NSS) * TB], f32, kind="ExternalInput").ap()
    ps_d = nc.dram_tensor("pst", [P, NPST], f32, kind="ExternalInput").ap()
    h_d = nc.dram_tensor("h", [P, HCOLS], outdt, kind="ExternalOutput").ap()

    # store offsets: long tiles then slot tiles
    soff = [0]
    for k in range(NSS):
        soff.append(soff[-1] + slot_lives[k])

    with tile.TileContext(nc) as tc:
        with (
            tc.tile_pool(name="const", bufs=1) as const,
            tc.tile_pool(name="wpool", bufs=1) as wpool,
            tc.tile_pool(name="xk", bufs=4) as xkpool,
            tc.tile_pool(name="elw", bufs=3) as elw,
            tc.tile_pool(name="spool", bufs=3) as spool,
            tc.tile_pool(name="psum", bufs=4, space="PSUM") as pspool,
        ):
            wk = []
            for k in range(KC):
                t = wpool.tile([P, EW], mmdt, tag=f"w{k}")
                nc.sync.dma_start(t[:], wT_d[k * P : (k + 1) * P, :])
                wk.append(t)
            av = const.tile([P, 1 + NSS], f32)
            nc.sync.dma_start(av[:], av_d[:])
            gb = const.tile([P, 1 + NSS], f32)
            nc.sync.dma_start(gb[:], gb_d[:])
            apw = const.tile([P, (1 + NSS) * TB], f32)
            nc.sync.dma_start(apw[:], ap_d[:])
            pst = const.tile([P, NPST], f32)
            nc.sync.dma_start(pst[:], ps_d[:])
            ones = const.tile([P, TB], f32)
            nc.vector.memset(ones[:], 1.0)
            abc = []
            for s in range(1 + NSS):
                t = const.tile([P, TB], f32, tag=f"abc{s}")
                nc.vector.tensor_scalar_mul(t[:], ones[:], av[:, s : s + 1])
                abc.append(t)

            st_eng = {"scalar": nc.scalar, "sync": nc.sync,
                      "gpsimd": nc.gpsimd, "vector": nc.vector}[STORE_ENG]

            def tilework(xkb, slot, pcol, hcol, init, chunk):
                """one [P,TB] tile: slot = param column (0=long, 1+k=slot k),
                pcol = pstart column, hcol = output tile index, init = scan
                init (0.0 or AP), chunk = xkb chunk holding own channels."""
                hs = slice(hcol * TB, (hcol + 1) * TB)
                ss = slice(slot * TB, (slot + 1) * TB)
                es = slice(slot * P, (slot + 1) * P)
                if mode == "dma":
                    st_eng.dma_start(h_d[:, hs], apw[:, :TB])
                    return None
                ps = pspool.tile([P, TB], f32)
                for k in range(KC):
                    nc.tensor.matmul(
                        ps[:], wk[k][:, es], xkb[:, k, :],
                        start=(k == 0), stop=(k == KC - 1),
                    )
                gt = elw.tile([P, TB], f32, tag="g")
                nc.scalar.activation(
                    gt[:], ps[:], Act.Sigmoid, bias=gb[:, slot : slot + 1],
                    scale=1.0,
                )
                pt = elw.tile([P, TB], f32, tag="p")
                nc.vector.tensor_scalar_mul(
                    pt[:], apw[:, ss], pst[:, pcol : pcol + 1]
                )
                dl = elw.tile([P, TB], f32, tag="dl")
                nc.gpsimd.tensor_scalar_add(dl[:], pt[:], KREG)
                q = elw.tile([P, TB], f32, tag="q")
                nc.vector.tensor_tensor(q[:], dl[:], dl[:], Alu.mult)
                q2 = elw.tile([P, TB], f32, tag="q2")
                nc.vector.tensor_scalar_add(q2[:], q[:], TAU2)
                rc = elw.tile([P, TB], f32, tag="rc")
                nc.vector.reciprocal_approx_fast(rc[:], q2[:])
                pd = elw.tile([P, TB], f32, tag="pd")
                nc.gpsimd.tensor_tensor(pd[:], pt[:], dl[:], Alu.mult)
                ft = elw.tile([P, TB], f32, tag="f")
                nc.vector.tensor_tensor(ft[:], pd[:], rc[:], Alu.mult)
                xg = elw.tile([P, TB], f32, tag="xg")
                nc.gpsimd.tensor_tensor(xg[:], gt[:], xkb[:, chunk, :], Alu.mult)
                wt = elw.tile([P, TB], f32, tag="w")
                nc.vector.tensor_tensor(wt[:], xg[:], ft[:], Alu.mult)
                st = spool.tile([P, TB], outdt, tag=f"s{slot}")
                nc.vector.tensor_tensor_scan(
                    st[:], abc[slot][:], wt[:], init, Alu.mult, Alu.add
                )
                if mode == "full":
                    st_eng.dma_start(h_d[:, hs], st[:])
                return st

            def body():
                prev_long = None
                prev_slot = [None] * NSS
                xkb_c = None
                for j in range(NLB):
                    # long-group block j (my T-half local block j)
                    if mode == "compute":
                        if xkb_c is None:
                            xkb_c = xkpool.tile([P, KC, TB], mmdt, tag="xkb")
                            nc.sync.dma_start(xkb_c[:], xp_d[:, 0])
                        xkb = xkb_c
                    else:
                        xkb = xkpool.tile([P, KC, TB], mmdt, tag="xkb")
                        eng = nc.sync if (not LOAD_SPLIT or j % 2 == 0) else nc.scalar
                        eng.dma_start(xkb[:], xp_d[:, j])
                    init = (
                        0.0
                        if j == 0 or prev_long is None
                        else prev_long[:, TB - 1 : TB]
                    )
                    prev_long = tilework(xkb, 0, j, j, init, 0)
                    # short slots, interleaved with the first nsb long blocks
                    if j < nsb:
                        if mode == "compute":
                            xkb_s = xkb_c
                        else:
                            xkb_s = xkpool.tile([P, KC, TB], mmdt, tag="xkb")
                            eng = nc.scalar if (not LOAD_SPLIT or j % 2 == 0) else nc.sync
                            eng.dma_start(xkb_s[:], xp_d[:, NLB + j])
                        for k in range(NSS):
                            if j >= slot_lives[k]:
                                continue
                            init = (
                                0.0
                                if j == 0 or prev_slot[k] is None
                                else prev_slot[k][:, TB - 1 : TB]
                            )
                            prev_slot[k] = tilework(
                                xkb_s, 1 + k, NLB + k * nsb + j,
                                NLB + soff[k] + j, init, 1 + k,
                            )

            if repeat == 1:
                body()
            else:
                with tc.For_i(0, repeat, 1):
                    body()
    nc.compile()
    return nc


# ---------------------------------------------------------------- host math
def _pows_table(a_safe):
    """fp32 sequential cumprod with FTZ, matching XLA bits."""
    a_rep = np.broadcast_to(a_safe, (T, D)).astype(np.float32)
    cp = np.cumprod(a_rep[1:], axis=0, dtype=np.float32)
    pows = np.concatenate([np.ones((1, D), np.float32), cp], axis=0)
    pows[np.abs(pows) < FP32_MIN_NORMAL] = 0.0
    return pows


# ---------------------------------------------------------------- kernel
def kernel(x, a_logit, b, c, d, gate_W, gate_b):
    from concourse.bass_utils import run_bass_kernel_spmd

    x = np.ascontiguousarray(np.asarray(x, np.float32))
    a_logit = np.asarray(a_logit, np.float32)
    b = np.asarray(b, np.float32)
    c = np.asarray(c, np.float32)
    d = np.asarray(d, np.float32)
    gate_W = np.ascontiguousarray(np.asarray(gate_W, np.float32))
    gate_b = np.asarray(gate_b, np.float32)

    a = np.tanh(a_logit)
    eps = np.float32(1e-4)
    a_safe = np.where(np.abs(a) < eps, np.where(a < 0, -eps, eps), a).astype(
        np.float32
    )
    pows = _pows_table(a_safe)

    order = np.argsort(-np.abs(a_safe), kind="stable")
    longch = order[:P]                                  # top-128 by |a|
    shorts = [order[P + k * P : P + (k + 1) * P] for k in range(7)]
    slives = []
    for ch in shorts:
        alive = (pows[:, ch] != 0).any(axis=1).reshape(NT, TB).any(axis=1)
        nz = np.nonzero(alive)[0]
        slives.append(int(nz.max()) + 1 if nz.size else 1)
    # role 0 slots: shorts 0,2,4,6 ; role 1 slots: 1,3,5, dummy(=6)
    role_groups = [[0, 2, 4, 6], [1, 3, 5, 6]]
    slot_lives = tuple(
        max(slives[role_groups[0][k]], slives[role_groups[1][k]])
        for k in range(NSS)
    )
    nsb = max(slot_lives)
    live = ("v3", slot_lives, nsb)

    key = (live, MM_DTYPE, OUT_DTYPE, LOAD_SPLIT, STORE_ENG)
    if key not in _prog_cache:
        _prog_cache[key] = _build_program(live)
    nc = _prog_cache[key]

    soff = [0]
    for k in range(NSS):
        soff.append(soff[-1] + slot_lives[k])

    in_maps = []
    for core in range(N_CORES):
        bb, role = divmod(core, 2)
        slotch = [shorts[gidx] for gidx in role_groups[role]]
        own = np.concatenate([longch] + slotch)          # 640 channels
        restset = np.setdiff1d(order, own, assume_unique=False)
        perm_full = np.concatenate([own, restset])       # (D,) permutation
        xTb = x[bb].T                                    # [D, T]
        xpr = _mm_cast(xTb[perm_full])                   # [D, T]
        # xpack slots: 0..NLB-1 = my long half blocks; NLB.. = global 0..nsb-1
        myblocks = [role * NLB + j for j in range(NLB)] + list(range(nsb))
        xp = np.ascontiguousarray(
            xpr.reshape(KC, P, NT, TB)[:, :, myblocks].transpose(1, 2, 0, 3)
        )                                                # [P, NSLOT, KC, TB]
        wT = _mm_cast(np.ascontiguousarray(gate_W[np.ix_(own, perm_full)].T))
        groups = [longch] + slotch                       # 5 x 128 channels
        avm = np.stack([a_safe[g] for g in groups], 1)   # [P, 5]
        gbm = np.stack([gate_b[g] for g in groups], 1)
        apwm = np.concatenate([pows[:TB, g].T for g in groups], 1)  # [P,5*TB]
        pcols = []
        for j in range(NLB):                             # long pstarts
            pcols.append(pows[(role * NLB + j) * TB, longch])
        for k in range(NSS):                             # slot pstarts
            for j in range(nsb):
                pcols.append(pows[j * TB, slotch[k]])
        pstm = np.stack(pcols, 1)                        # [P, NPST]
        in_maps.append(
            {
                "xp": xp,
                "wT": wT,
                "av": np.ascontiguousarray(avm.astype(np.float32)),
                "gbv": np.ascontiguousarray(gbm.astype(np.float32)),
                "apw": np.ascontiguousarray(apwm.astype(np.float32)),
                "pst": np.ascontiguousarray(pstm.astype(np.float32)),
            }
        )

    global last_results, last_live, last_in_maps
    last_live = live
    last_in_maps = in_maps
    res = run_bass_kernel_spmd(nc, in_maps, core_ids=list(range(N_CORES)))
    last_results = res

    h = np.zeros((B, T, D), np.float32)
    decay = pows[1 : T // 2 + 1, longch]                 # a^(t-4095), t=4096..8191
    for core in range(N_CORES):
        bb, role = divmod(core, 2)
        hr = res.results[core]["h"].astype(np.float32)   # [P, HCOLS]
        hl = hr[:, : NLB * TB].T                         # [T/2, 128] long part
        if role == 0:
            h[bb][: T // 2, longch] = hl
        else:
            # rank-1 state-handoff correction: s4095 from role 0's output
            h0 = res.results[core - 1]["h"].astype(np.float32)
            s4095 = h0[:, NLB * TB - 1]                  # [128]
            h[bb][T // 2 :, longch] = hl + decay * s4095[None, :]
        slotch = [shorts[gidx] for gidx in role_groups[role]]
        for k in range(NSS):
            if role == 1 and k == NSS - 1:
                continue                                 # dummy slot
            cols = slice((NLB + soff[k]) * TB, (NLB + soff[k + 1]) * TB)
            h[bb][: slot_lives[k] * TB, slotch[k]] = hr[:, cols].T
    cb = (c * b).astype(np.float32)
    if np.any(cb != 1):
        h *= cb[None, None, :]
    if np.any(d != 0):
        h += d[None, None, :] * x
    return h


last_results = None


# revision 11
# speedup vs baseline: 4.5083x; 3.9102x over previous
"""Trainium2 Bass kernel for nn_DiagonalSelectiveSSM.

Math (reference):
    a = tanh(a_logit); a_safe = sign-clamped to |a|>=1e-4
    g = sigmoid(x @ W^T + gate_b)
    u = b * g * x
    pows[t] = cumprod(a_safe) (fp32, underflows to exact 0 under FTZ)
    v = u / (pows + 1e-12); s = cumsum(v) * pows; h = c*s + d*x

Identity: s_t = a_safe * s_{t-1} + w_t  with  w_t = u_t * F_t and
F_t = pows_t / (pows_t + 1e-12). For a<0 channels F has catastrophic-
cancellation *spikes* (|F| up to ~1e4 where pows ~ -1e-12) which dominate
the output norm, so F is rebuilt on device from the exact fp32 cumprod
bits:  p = pstart[slot,j] * apow[tau],  F = p*(p+K)/((p+K)^2 + tau^2)
(zero-safe soft reciprocal; exact to ~1e-6 away from the poles).

Sharding (8 cores = 4 sequences x 2 T-halves):
  The output norm is dominated by the ~128 longest-lived channels (their
  variance is amplified by 1/(1-a^2)), and they force every t-block of x
  to be read for their gates. So per sequence the pair of cores splits T:
    role 0: long group x global blocks 0..7,  role 1: blocks 8..15.
  Both scan from state 0; the reference state at the half boundary is
  role 0's last stored column, and the host adds the exact rank-1 decay
  correction  s_true = s~ + a^(t-4095) * s(4095)  (linear recurrence)
  to role 1's long tiles during the gather pass - the chunked state
  handoff of T-sharding, applied as a host epilogue.
  The remaining 896 channels (short-lived: dead after <=2 blocks) are
  dealt between the pair as 4 slot-groups each (one is a balance dummy),
  reading the same early global blocks from dedicated xpack slots.
  Dead tiles are skipped entirely (runtime pre-zeros outputs).
b/c/d generality is a host epilogue (s is linear in u).

x ships once per core as fp16, host-packed to [P, slot, KC*TB] so each
block load is 128 partitions x 8KB contiguous; contraction chunk order
puts each slot's output channels in one chunk so the elementwise x is
reused from the matmul tile (no separate x stream).
"""

import os

import numpy as np

B, T, D = 4, 8192, 1024
P = 128             # partitions
NLB = 8             # long-group blocks per core (T/2 / TB)
NSS = 4             # short slots per core
TB = 512            # time-block (one PSUM bank of fp32)
NT = T // TB
KC = D // P         # contraction chunks
EW = P + NSS * P    # gate output channels per core (long + 4 slots) = 640
N_CORES = 8
FP32_MIN_NORMAL = np.float32(1.1754944e-38)
KREG = 1e-12
TAU2 = float(np.float32(1e-17) ** 2)
MM_DTYPE = os.environ.get("KERNEL_MM_DTYPE", "fp16")
OUT_DTYPE = os.environ.get("KERNEL_OUT_DTYPE", "f32")
LOAD_SPLIT = int(os.environ.get("KERNEL_LOAD_SPLIT", "1"))
STORE_ENG = os.environ.get("KERNEL_STORE_ENG", "scalar")

_prog_cache = {}


def _mm_cast(a):
    if MM_DTYPE == "f32r":
        b = np.ascontiguousarray(a, np.float32).view(np.uint32)
        lsb = (b >> 12) & 1
        return ((b + 0x7FF + lsb) & 0xFFFFF000).view(np.float32)
    if MM_DTYPE == "fp16":
        return np.ascontiguousarray(a).astype(np.float16)
    import ml_dtypes

    return np.ascontiguousarray(a).astype(ml_dtypes.bfloat16)


# ---------------------------------------------------------------- program
def _build_program(live, repeat=1, mode="full"):
    """live: (v3, slot_lives tuple(NSS), nsb). One program for all 8 cores;
    role differences are pure input data. repeat>1 wraps the body in a
    hardware loop (benchmarking). mode: full | dma | compute."""
    import concourse.tile as tile
    from concourse import bacc, mybir

    _tag, slot_lives, nsb = live
    f32 = mybir.dt.float32
    mmdt = {
        "f32r": mybir.dt.float32r,
        "fp16": mybir.dt.float16,
        "bf16": mybir.dt.bfloat16,
    }[MM_DTYPE]
    outdt = f32 if OUT_DTYPE == "f32" else mybir.dt.float16
    Alu = mybir.AluOpType
    Act = mybir.ActivationFunctionType

    NSLOT = NLB + nsb               # xpack slots
    NPST = NLB + NSS * nsb          # pstart columns
    HCOLS = (NLB + sum(slot_lives)) * TB

    nc = bacc.Bacc(
        "TRN2",
        target_bir_lowering=False,
        debug=False,
        enable_asserts=False,
        num_devices=N_CORES,
    )

    xp_d = nc.dram_tensor("xp", [P, NSLOT, KC, TB], mmdt, kind="ExternalInput").ap()
    wT_d = nc.dram_tensor("wT", [D, EW], mmdt, kind="ExternalInput").ap()
    av_d = nc.dram_tensor("av", [P, 1 + NSS], f32, kind="ExternalInput").ap()
    gb_d = nc.dram_tensor("gbv", [P, 1 + NSS], f32, kind="ExternalInput").ap()
    ap_d = nc.dram_tensor("apw", [P, (1 + NSS) * TB], f32, kind="ExternalInput").ap()
    ps_d = nc.dram_tensor("pst", [P, NPST], f32, kind="ExternalInput").ap()
    h_d = nc.dram_tensor("h", [P, HCOLS], outdt, kind="ExternalOutput").ap()

    # store offsets: long tiles then slot tiles
    soff = [0]
    for k in range(NSS):
        soff.append(soff[-1] + slot_lives[k])

    with tile.TileContext(nc) as tc:
        with (
            tc.tile_pool(name="const", bufs=1) as const,
            tc.tile_pool(name="wpool", bufs=1) as wpool,
            tc.tile_pool(name="xk", bufs=4) as xkpool,
            tc.tile_pool(name="elw", bufs=3) as elw,
            tc.tile_pool(name="spool", bufs=3) as spool,
            tc.tile_pool(name="psum", bufs=4, space="PSUM") as pspool,
        ):
            wk = []
            for k in range(KC):
                t = wpool.tile([P, EW], mmdt, tag=f"w{k}")
                nc.sync.dma_start(t[:], wT_d[k * P : (k + 1) * P, :])
                wk.append(t)
            av = const.tile([P, 1 + NSS], f32)
            nc.sync.dma_start(av[:], av_d[:])
            gb = const.tile([P, 1 + NSS], f32)
            nc.sync.dma_start(gb[:], gb_d[:])
            apw = const.tile([P, (1 + NSS) * TB], f32)
            nc.sync.dma_start(apw[:], ap_d[:])
            pst = const.tile([P, NPST], f32)
            nc.sync.dma_start(pst[:], ps_d[:])
            ones = const.tile([P, TB], f32)
            nc.vector.memset(ones[:], 1.0)
            abc = []
            for s in range(1 + NSS):
                t = const.tile([P, TB], f32, tag=f"abc{s}")
                nc.vector.tensor_scalar_mul(t[:], ones[:], av[:, s : s + 1])
                abc.append(t)

            st_eng = {"scalar": nc.scalar, "sync": nc.sync,
                      "gpsimd": nc.gpsimd, "vector": nc.vector}[STORE_ENG]

            def tilework(xch, slot, pcol, hcol, init, chunk):
                """one [P,TB] tile: slot = param column (0=long, 1+k=slot k),
                pcol = pstart column, hcol = output tile index, init = scan
                init (0.0 or AP), chunk = xkb chunk holding own channels."""
                hs = slice(hcol * TB, (hcol + 1) * TB)
                ss = slice(slot * TB, (slot + 1) * TB)
                es = slice(slot * P, (slot + 1) * P)
                if mode == "dma":
                    st_eng.dma_start(h_d[:, hs], apw[:, :TB])
                    return None
                ps = pspool.tile([P, TB], f32)
                for k in range(KC):
                    nc.tensor.matmul(
                        ps[:], wk[k][:, es], xch(k),
                        start=(k == 0), stop=(k == KC - 1),
                    )
                gt = elw.tile([P, TB], f32, tag="g")
                nc.scalar.activation(
                    gt[:], ps[:], Act.Sigmoid, bias=gb[:, slot : slot + 1],
                    scale=1.0,
                )
                pt = elw.tile([P, TB], f32, tag="p")
                nc.vector.tensor_scalar_mul(
                    pt[:], apw[:, ss], pst[:, pcol : pcol + 1]
                )
                dl = elw.tile([P, TB], f32, tag="dl")
                nc.scalar.activation(dl[:], pt[:], Act.Copy, bias=KREG, scale=1.0)
                q = elw.tile([P, TB], f32, tag="q")
                nc.vector.tensor_tensor(q[:], dl[:], dl[:], Alu.mult)
                q2 = elw.tile([P, TB], f32, tag="q2")
                nc.vector.tensor_scalar_add(q2[:], q[:], TAU2)
                rc = elw.tile([P, TB], f32, tag="rc")
                nc.vector.reciprocal_approx_fast(rc[:], q2[:])
                pd = elw.tile([P, TB], f32, tag="pd")
                nc.vector.tensor_tensor(pd[:], pt[:], dl[:], Alu.mult)
                ft = elw.tile([P, TB], f32, tag="f")
                nc.vector.tensor_tensor(ft[:], pd[:], rc[:], Alu.mult)
                xg = elw.tile([P, TB], f32, tag="xg")
                nc.vector.tensor_tensor(xg[:], gt[:], xch(chunk), Alu.mult)
                wt = elw.tile([P, TB], f32, tag="w")
                nc.vector.tensor_tensor(wt[:], xg[:], ft[:], Alu.mult)
                st = spool.tile([P, TB], outdt, tag=f"s{slot}")
                nc.vector.tensor_tensor_scan(
                    st[:], abc[slot][:], wt[:], init, Alu.mult, Alu.add
                )
                if mode == "full":
                    st_eng.dma_start(h_d[:, hs], st[:])
                return st

            def body():
                prev_long = None
                prev_slot = [None] * NSS
                nload = [0]

                def load_merge(s0, n):
                    t = xkpool.tile([P, n, KC, TB], mmdt, tag="xkb")
                    eng = (
                        nc.sync
                        if (not LOAD_SPLIT or nload[0] % 2 == 0)
                        else nc.scalar
                    )
                    nload[0] += 1
                    eng.dma_start(t[:], xp_d[:, s0 : s0 + n])
                    return t

                xkb_c = load_merge(0, 1) if mode == "compute" else None
                for jp in range(0, NLB, 2):
                    n = min(2, NLB - jp)
                    xkb2 = xkb_c if mode == "compute" else load_merge(jp, n)
                    for jj in range(n):
                        j = jp + jj
                        ji = 0 if mode == "compute" else jj
                        xch = (lambda t, i: (lambda k: t[:, i, k, :]))(xkb2, ji)
                        init = (
                            0.0
                            if j == 0 or prev_long is None
                            else prev_long[:, TB - 1 : TB]
                        )
                        prev_long = tilework(xch, 0, j, j, init, 0)
                    if jp == 0:
                        # short slots read global blocks 0..nsb-1
                        xkbs = (
                            xkb_c if mode == "compute" else load_merge(NLB, nsb)
                        )
                        for j2 in range(nsb):
                            ji = 0 if mode == "compute" else j2
                            xch = (lambda t, i: (lambda k: t[:, i, k, :]))(xkbs, ji)
                            for k in range(NSS):
                                if j2 >= slot_lives[k]:
                                    continue
                                init = (
                                    0.0
                                    if j2 == 0 or prev_slot[k] is None
                                    else prev_slot[k][:, TB - 1 : TB]
                                )
                                prev_slot[k] = tilework(
                                    xch, 1 + k, NLB + k * nsb + j2,
                                    NLB + soff[k] + j2, init, 1 + k,
                                )

            if repeat == 1:
                body()
            else:
                with tc.For_i(0, repeat, 1):
                    body()
    nc.compile()
    return nc


# ---------------------------------------------------------------- host math
def _pows_table(a_safe):
    """fp32 sequential cumprod with FTZ, matching XLA bits."""
    a_rep = np.broadcast_to(a_safe, (T, D)).astype(np.float32)
    cp = np.cumprod(a_rep[1:], axis=0, dtype=np.float32)
    pows = np.concatenate([np.ones((1, D), np.float32), cp], axis=0)
    pows[np.abs(pows) < FP32_MIN_NORMAL] = 0.0
    return pows


# ---------------------------------------------------------------- kernel
def kernel(x, a_logit, b, c, d, gate_W, gate_b):
    from concourse.bass_utils import run_bass_kernel_spmd

    x = np.ascontiguousarray(np.asarray(x, np.float32))
    a_logit = np.asarray(a_logit, np.float32)
    b = np.asarray(b, np.float32)
    c = np.asarray(c, np.float32)
    d = np.asarray(d, np.float32)
    gate_W = np.ascontiguousarray(np.asarray(gate_W, np.float32))
    gate_b = np.asarray(gate_b, np.float32)

    a = np.tanh(a_logit)
    eps = np.float32(1e-4)
    a_safe = np.where(np.abs(a) < eps, np.where(a < 0, -eps, eps), a).astype(
        np.float32
    )
    pows = _pows_table(a_safe)

    order = np.argsort(-np.abs(a_safe), kind="stable")
    longch = order[:P]                                  # top-128 by |a|
    shorts = [order[P + k * P : P + (k + 1) * P] for k in range(7)]
    slives = []
    for ch in shorts:
        alive = (pows[:, ch] != 0).any(axis=1).reshape(NT, TB).any(axis=1)
        nz = np.nonzero(alive)[0]
        slives.append(int(nz.max()) + 1 if nz.size else 1)
    # role 0 slots: shorts 0,2,4,6 ; role 1 slots: 1,3,5, dummy(=6)
    role_groups = [[0, 2, 4, 6], [1, 3, 5, 6]]
    slot_lives = tuple(
        max(slives[role_groups[0][k]], slives[role_groups[1][k]])
        for k in range(NSS)
    )
    nsb = max(slot_lives)
    live = ("v3", slot_lives, nsb)

    key = (live, MM_DTYPE, OUT_DTYPE, LOAD_SPLIT, STORE_ENG)
    if key not in _prog_cache:
        _prog_cache[key] = _build_program(live)
    nc = _prog_cache[key]

    soff = [0]
    for k in range(NSS):
        soff.append(soff[-1] + slot_lives[k])

    in_maps = []
    for core in range(N_CORES):
        bb, role = divmod(core, 2)
        slotch = [shorts[gidx] for gidx in role_groups[role]]
        own = np.concatenate([longch] + slotch)          # 640 channels
        restset = np.setdiff1d(order, own, assume_unique=False)
        perm_full = np.concatenate([own, restset])       # (D,) permutation
        xTb = x[bb].T                                    # [D, T]
        xpr = _mm_cast(xTb[perm_full])                   # [D, T]
        # xpack slots: 0..NLB-1 = my long half blocks; NLB.. = global 0..nsb-1
        myblocks = [role * NLB + j for j in range(NLB)] + list(range(nsb))
        xp = np.ascontiguousarray(
            xpr.reshape(KC, P, NT, TB)[:, :, myblocks].transpose(1, 2, 0, 3)
        )                                                # [P, NSLOT, KC, TB]
        wT = _mm_cast(np.ascontiguousarray(gate_W[np.ix_(own, perm_full)].T))
        groups = [longch] + slotch                       # 5 x 128 channels
        avm = np.stack([a_safe[g] for g in groups], 1)   # [P, 5]
        gbm = np.stack([gate_b[g] for g in groups], 1)
        apwm = np.concatenate([pows[:TB, g].T for g in groups], 1)  # [P,5*TB]
        pcols = []
        for j in range(NLB):                             # long pstarts
            pcols.append(pows[(role * NLB + j) * TB, longch])
        for k in range(NSS):                             # slot pstarts
            for j in range(nsb):
                pcols.append(pows[j * TB, slotch[k]])
        pstm = np.stack(pcols, 1)                        # [P, NPST]
        in_maps.append(
            {
                "xp": xp,
                "wT": wT,
                "av": np.ascontiguousarray(avm.astype(np.float32)),
                "gbv": np.ascontiguousarray(gbm.astype(np.float32)),
                "apw": np.ascontiguousarray(apwm.astype(np.float32)),
                "pst": np.ascontiguousarray(pstm.astype(np.float32)),
            }
        )

    global last_results, last_live, last_in_maps
    last_live = live
    last_in_maps = in_maps
    res = run_bass_kernel_spmd(nc, in_maps, core_ids=list(range(N_CORES)))
    last_results = res

    h = np.zeros((B, T, D), np.float32)
    decay = pows[1 : T // 2 + 1, longch]                 # a^(t-4095), t=4096..8191
    for core in range(N_CORES):
        bb, role = divmod(core, 2)
        hr = res.results[core]["h"].astype(np.float32)   # [P, HCOLS]
        hl = hr[:, : NLB * TB].T                         # [T/2, 128] long part
        if role == 0:
            h[bb][: T // 2, longch] = hl
        else:
            # rank-1 state-handoff correction: s4095 from role 0's output
            h0 = res.results[core - 1]["h"].astype(np.float32)
            s4095 = h0[:, NLB * TB - 1]                  # [128]
            h[bb][T // 2 :, longch] = hl + decay * s4095[None, :]
        slotch = [shorts[gidx] for gidx in role_groups[role]]
        for k in range(NSS):
            if role == 1 and k == NSS - 1:
                continue                                 # dummy slot
            cols = slice((NLB + soff[k]) * TB, (NLB + soff[k + 1]) * TB)
            h[bb][: slot_lives[k] * TB, slotch[k]] = hr[:, cols].T
    cb = (c * b).astype(np.float32)
    if np.any(cb != 1):
        h *= cb[None, None, :]
    if np.any(d != 0):
        h += d[None, None, :] * x
    return h


last_results = None
